# revision 51
# baseline (speedup 1.0000x reference)
"""DIN-style attention layer on 8 Trainium2 NeuronCores.

Problem: q[B,64], k[B,200,64], v[B,200,64], mask[B,200]; per-token MLP on
DIN features concat([q,k,q-k,q*k]) -> 80 -> 40 -> 1 logits, masked softmax
over T, then attn-weighted sum of v. B=2048 sharded over 8 cores.

Math refactor (host):
  info@W1 = q@(W1a+W1c) + k@(W1b-W1c) + (q*k)@W1d   with W1=[W1a;W1b;W1c;W1d]
  => h1_b = relu( Wb_eff^T kt_b + beta_b ),  Wb_eff = (W1b-W1c) + q_b*W1d
bf is dropped: softmax is shift-invariant.

Device design (per core, 256 batches = 128 pairs = 16 groups of 8 pairs):
  68 features per (b,t): [k(64), ones, ones, mask01, 0]; per-batch fused
  input comb[68, b, 282] fp8 = [kt stream (200) | weights (82)]. Weight
  cols 0:80 = h1 units (q and beta folded in; beta split across the two
  ones rows for fp8 precision), col 80 selects mask01, col 81 const-1.
  L1/batch: psum[82,200] fp8 matmul (contract 68). relu -> H1S bf16 (rows
  80/81 pass mask01/1 through). L2/pair: w2p[82,64] bf16 -> psum rows
  {0-39,64-103} (+mask/one rows 40,41,104,105). relu+b2 (ACT bias AP).
  L3/chunk: wfbd[106,32] with +-30 on mask/one rows -> pre-masked logits
  in sparse rows {32jj+i} of psum[128,432]. exp (ACT) -> EX bf16;
  PE-transpose (100-col chunks); out = v^T @ attn^T per pair and exp-sums
  via a ones-row matmul, all accumulated into psum cols 400:432; copied
  once per group to SBUF and DMA'd out once at the end. Engine placement:
  GPSIMD cannot touch PSUM on TRN2, so relus run DVE 3:1 ACT; emission is
  a 3-deep software pipeline over groups (fp8 DoubleRow measured ~2.4x
  slower than plain fp8 on HW and is not used).
"""

import os
import sys

import numpy as np

for _p in ("/opt/trn_rl_repo", "/root/.axon_site/_ro/trn_rl_repo"):
    if os.path.isdir(_p) and _p not in sys.path:
        sys.path.insert(0, _p)

import ml_dtypes

BF16 = ml_dtypes.bfloat16
FP8 = ml_dtypes.float8_e4m3

B, T, D = 2048, 200, 64
H1, H2 = 80, 40
NCORES = 8
BC = B // NCORES          # 256 batches per core
PAIRS = BC // 2           # 128
NG = PAIRS // 8           # 16 groups of 8 pairs (16 batches)
GB = 16                   # batches per group
# Mask additive constant. Must be small enough that f32 PSUM accumulation
# through +BIGM*mask01 - BIGM preserves the ~+-0.06 logits (ulp(30)=1.9e-6),
# and big enough that exp(logit-BIGM) ~ 1e-13 vanishes vs exp(logit) ~ 1.
BIGM = 30.0


def _build_bass():
    from concourse import bass, bacc, tile
    from concourse import mybir

    STAGE = int(os.environ.get("KSTAGE", "9"))
    KLOOP = int(os.environ.get("KLOOP", "1"))
    KBF16 = int(os.environ.get("KBF16", "2"))

    dt = mybir.dt
    DR = mybir.MatmulPerfMode.DoubleRow
    nc = bacc.Bacc("TRN2", target_bir_lowering=False, debug=False)

    if KBF16 == 1:
        comb = nc.declare_dram_parameter("comb", [68, BC, 282], dt.bfloat16, False)
    elif KBF16 == 2:
        comb = nc.declare_dram_parameter("comb", [68, BC, 282], dt.float8e4, False)
    else:
        comb = nc.declare_dram_parameter("comb", [34, 2, BC, 282], dt.float8e4, False)
    v2d = nc.declare_dram_parameter("v2d", [100, PAIRS, 2, 128], dt.bfloat16, False)
    w2p = nc.declare_dram_parameter("w2p", [82, 64], dt.bfloat16, False)
    wfbd = nc.declare_dram_parameter("wfbd", [106, 32], dt.bfloat16, False)
    b2s = nc.declare_dram_parameter("b2s", [128, 1], dt.float32, False)
    ident = nc.declare_dram_parameter("ident", [128, 128], dt.bfloat16, False)
    outp = nc.declare_dram_parameter("outp", [128, NG, 32], dt.float32, True)

    with tile.TileContext(nc) as tc:
        with (
            tc.tile_pool(name="consts", bufs=1) as cpool,
            tc.tile_pool(name="cin", bufs=3) as cbpool,
            tc.tile_pool(name="vin", bufs=4) as vpool,
            tc.tile_pool(name="h1", bufs=8) as h1pool,
            tc.tile_pool(name="h2", bufs=4) as h2pool,
            tc.tile_pool(name="ex", bufs=3) as expool,
            tc.tile_pool(name="ats", bufs=2) as atspool,
            tc.tile_pool(name="ph1", bufs=2, space="PSUM") as ph1pool,
            tc.tile_pool(name="ph2", bufs=2, space="PSUM") as ph2pool,
            tc.tile_pool(name="plg", bufs=2, space="PSUM") as plgpool,
        ):
            w2_t = cpool.tile([82, 64], dt.bfloat16)
            nc.sync.dma_start(w2_t[:], w2p[:])
            wfbd_t = cpool.tile([106, 32], dt.bfloat16)
            nc.sync.dma_start(wfbd_t[:], wfbd[:])
            b2s_t = cpool.tile([128, 1], dt.float32)
            nc.sync.dma_start(b2s_t[:], b2s[:])
            id_t = cpool.tile([128, 128], dt.bfloat16)
            nc.sync.dma_start(id_t[:], ident[:])
            # keep engine clocks warm past const DMAs (see v1 note on
            # single sync-wait slots for AP-scalar ops)
            warm = cpool.tile([128, 1], dt.float32)
            nc.vector.tensor_copy(warm[:], b2s_t[:])
            ones_t = cpool.tile([128, 1], dt.bfloat16)
            nc.vector.memset(ones_t[:], 1.0)
            # per-group out-blocks (16 outs + 16 exp-sums), flushed at the end
            outall = cpool.tile([128, NG, 32], dt.float32)
            if STAGE < 9:  # ablation mode: ensure outputs have a writer
                nc.vector.memset(outall[:], 0.0)

            # ---- software-pipelined emission -------------------------------
            # Per group g, work items: B(jj)=L1x4+relu, C(jj)=L2x2+h2relu,
            # D(jj)=L3, E=exp, F=transposes, G=ATS copy, H=out-mms, R=rowsum,
            # I=outall copy. Three-deep group pipeline: round r runs B(r),
            # C/D/E/F(r-1), G/H/I/R(r-2), so every PSUM round-trip has a full
            # group period of slack and the in-order engine queues never park
            # on a dependency while independent work waits behind it. h1
            # activations buffer in SBUF (h1 pool bufs=16) across the phase
            # boundary.
            st = {"relu_idx": 0}
            ctx: dict = {}

            def emit_B(cur, jj):
                CB = cur["CB"]
                # one 2-bank PSUM tile per chunk (pair i at col i*512, batch
                # jb at +jb*200); a single relu covers both pairs. Cols
                # 400:512 are uninitialized psum and never read downstream.
                PH1 = ph1pool.tile([82, 1024], dt.float32)
                for i in range(2):
                    for jb in range(2):
                        bi = 4 * jj + 2 * i + jb
                        c0 = i * 512 + jb * 200
                        if KBF16:
                            nc.tensor.matmul(
                                PH1[:, c0 : c0 + 200],
                                lhsT=CB[:, bi, 200:282],
                                rhs=CB[:, bi, 0:200],
                                start=True,
                                stop=True,
                            )
                        else:
                            nc.tensor.matmul(
                                PH1[:, c0 : c0 + 200],
                                lhsT=CB[:, :, bi, 200:282],
                                rhs=CB[:, :, bi, 0:200],
                                start=True,
                                stop=True,
                                perf_mode=DR,
                            )
                H1S = h1pool.tile([82, 912], dt.bfloat16)
                # GPSIMD cannot access PSUM: relus split DVE 3 : ACT 1
                m = "VAVV"[st["relu_idx"] % 4]
                st["relu_idx"] += 1
                if m == "V":
                    nc.vector.tensor_scalar_max(H1S[:], PH1[:, 0:912], 0.0)
                else:
                    nc.scalar.activation(
                        H1S[:], PH1[:, 0:912],
                        mybir.ActivationFunctionType.Relu,
                    )
                cur[("H1", jj)] = H1S

            def emit_C(cur, jj):
                if STAGE < 2:
                    return
                PH2 = ph2pool.tile([128, 400], dt.float32, tag="PH2")
                for i in range(2):
                    nc.tensor.matmul(
                        PH2[64 * i : 64 * i + 64, :],
                        lhsT=w2_t[:],
                        rhs=cur[("H1", jj)][:, 512 * i : 512 * i + 400],
                        start=True,
                        stop=True,
                        tile_position=(0, 64 * i),
                    )
                H2S = h2pool.tile([106, 400], dt.bfloat16)
                nc.scalar.activation(
                    H2S[:],
                    PH2[0:106, :],
                    mybir.ActivationFunctionType.Relu,
                    bias=b2s_t[0:106, :],
                )
                cur[("H2", jj)] = H2S

            def emit_D(cur, jj):
                if STAGE < 3:
                    return
                if "PLG" not in cur:
                    PLG = plgpool.tile([128, 432], dt.float32)
                    cur["PLG"] = PLG
                nc.tensor.matmul(
                    cur["PLG"][32 * jj : 32 * jj + 32, 0:400],
                    lhsT=wfbd_t[:],
                    rhs=cur[("H2", jj)][:],
                    start=True,
                    stop=True,
                    tile_position=(0, 32 * jj),
                )

            def emit_E(p):
                if STAGE < 4 or "PLG" not in p:
                    return
                EX = expool.tile([128, 2, 200], dt.bfloat16)
                nc.scalar.activation(
                    EX[:], p["PLG"][:, 0:400],
                    mybir.ActivationFunctionType.Exp,
                )
                p["EX"] = EX

            def emit_F(p):
                if STAGE < 5 or "EX" not in p:
                    return
                PT = ph2pool.tile(
                    [100, 2, 2, 128], dt.bfloat16, name="PT", tag="PH2"
                )
                for jb in range(2):
                    for c in range(2):
                        nc.tensor.transpose(
                            PT[:, jb, c, :],
                            p["EX"][:, jb, c * 100 : (c + 1) * 100],
                            id_t[:],
                        )
                p["PT"] = PT

            def emit_G(p):
                if STAGE < 7 or "PT" not in p:
                    return
                ATS = atspool.tile([100, 2, 2, 4, 32], dt.bfloat16)
                if p["g"] % 2 == 0:
                    nc.vector.tensor_copy(ATS[:], p["PT"][:])
                else:
                    nc.scalar.activation(
                        ATS[:], p["PT"][:], mybir.ActivationFunctionType.Copy
                    )
                p["ATS"] = ATS

            def emit_H(p, qs):
                if STAGE < 8 or "ATS" not in p:
                    return
                for q in qs:
                    jj, i = q // 2, q % 2
                    for c in range(2):
                        nc.tensor.matmul(
                            p["PLG"][:, 400 + 2 * q : 402 + 2 * q],
                            lhsT=p["V2"][:, q, c, :],
                            rhs=p["ATS"][:, :, c, jj, i],
                            start=(c == 0),
                            stop=(c == 1),
                        )
                if 7 in qs:
                    # exp-sums on PE: ones^T @ attn^T for all 16 batches
                    for c in range(2):
                        nc.tensor.matmul(
                            p["PLG"][0:1, 416:432],
                            lhsT=ones_t[0:100, :],
                            rhs=p["ATS"][:, :, c, :, 0:2],
                            start=(c == 0),
                            stop=(c == 1),
                        )

            def emit_I(p):
                if STAGE < 9 or "PLG" not in p:
                    return
                nc.vector.tensor_copy(outall[:, p["g"], :], p["PLG"][:, 400:432])

            def body():
                for r in range(NG + 2):
                    if r < NG:
                        if KBF16:
                            cbdt = dt.bfloat16 if KBF16 == 1 else dt.float8e4
                            CB = cbpool.tile([68, GB, 282], cbdt)
                            nc.sync.dma_start(
                                CB[:], comb[:, r * GB : (r + 1) * GB, :]
                            )
                        else:
                            CB = cbpool.tile([34, 2, GB, 282], dt.float8e4)
                            nc.sync.dma_start(
                                CB[:], comb[:, :, r * GB : (r + 1) * GB, :]
                            )
                        V2 = vpool.tile([100, 8, 2, 128], dt.bfloat16)
                        nc.sync.dma_start(V2[:], v2d[:, r * 8 : (r + 1) * 8, :, :])
                        ctx[r] = {"g": r, "CB": CB, "V2": V2}
                    p1 = ctx.get(r - 1)
                    p2 = ctx.get(r - 2)

                    if r < NG:
                        emit_B(ctx[r], 0)
                    if p2:
                        emit_F(p2)
                        emit_G(p2)
                    if r < NG:
                        emit_B(ctx[r], 1)
                    if p2:
                        emit_H(p2, range(0, 8))
                    if p1:
                        emit_C(p1, 0)
                    if p2:
                        emit_I(p2)
                        del ctx[r - 2]
                    if r < NG:
                        emit_B(ctx[r], 2)
                    if p1:
                        emit_C(p1, 1)
                    if r < NG:
                        emit_B(ctx[r], 3)
                    if p1:
                        emit_C(p1, 2)
                        emit_C(p1, 3)
                        emit_D(p1, 0)
                        emit_D(p1, 1)
                        emit_D(p1, 2)
                        emit_D(p1, 3)
                        emit_E(p1)

                nc.sync.dma_start(outp[:], outall[:])

            if KLOOP > 1:
                with tc.For_i(0, KLOOP):
                    body()
            else:
                body()

    nc.compile()
    return nc


_NC_CACHE = {}


def _get_nc():
    if "nc" not in _NC_CACHE:
        _NC_CACHE["nc"] = _build_bass()
    return _NC_CACHE["nc"]


def _prep_core(qc, kc, vc, mc, W1, b1, W2, b2, Wf):
    """Build the per-core DRAM input dict (numpy, host-side)."""
    f32 = np.float32
    W1a, W1b_, W1c, W1d = W1[0:64], W1[64:128], W1[128:192], W1[192:256]

    # features [68, BC, T]: 0:64 k^T, 64/65 ones, 66 mask01, 67 pad
    feats = np.zeros((68, BC, T), dtype=f32)
    feats[0:64] = kc.transpose(2, 0, 1)
    feats[64] = 1.0
    feats[65] = 1.0
    feats[66] = mc.astype(f32)

    # weights [68, BC, 82]
    wb_eff = (W1b_ - W1c)[None, :, :] + qc[:, :, None] * W1d[None, :, :]
    beta = qc @ (W1a + W1c) + b1[None, :]
    beta_hi = beta.astype(FP8).astype(f32)
    beta_lo = beta - beta_hi
    w68 = np.zeros((68, BC, 82), dtype=f32)
    w68[0:64, :, 0:80] = wb_eff.transpose(1, 0, 2)
    w68[64, :, 0:80] = beta_hi
    w68[65, :, 0:80] = beta_lo
    w68[66, :, 80] = 1.0  # mask01 pass-through
    w68[64, :, 81] = 1.0  # const-1 pass-through

    kb = os.environ.get("KBF16", "2")
    if kb in ("1", "2"):
        cdt = BF16 if kb == "1" else FP8
        comb = np.empty((68, BC, 282), dtype=cdt)
        comb[:, :, 0:200] = feats.astype(cdt)
        comb[:, :, 200:282] = w68.astype(cdt)
    else:
        # comb [34, 2, BC, 282] fp8: planes are feature blocks [0:34), [34:68)
        comb = np.empty((34, 2, BC, 282), dtype=FP8)
        comb[:, :, :, 0:200] = (
            feats.reshape(2, 34, BC, T).transpose(1, 0, 2, 3).astype(FP8)
        )
        comb[:, :, :, 200:282] = (
            w68.reshape(2, 34, BC, 82).transpose(1, 0, 2, 3).astype(FP8)
        )

    # v2d [100, PAIRS, 2, 128]: [t%100, pair, t//100, jb*64+d], dense
    v2d = np.ascontiguousarray(
        vc.reshape(PAIRS, 2, 2, 100, D)
        .transpose(3, 0, 2, 1, 4)
        .reshape(100, PAIRS, 2, 128)
    ).astype(BF16)

    # w2p [82, 64]: W2 at [0:80, 0:40]; selectors pass mask/const through L2
    w2p = np.zeros((82, 64), dtype=BF16)
    w2p[0:80, 0:40] = W2.astype(BF16)
    w2p[80, 40] = 1.0
    w2p[81, 41] = 1.0

    # wfbd [106, 32]: Wf per pair-column + mask fold (+BIG*mask01 - BIG)
    big = np.float32(BIGM)
    wfbd = np.zeros((106, 32), dtype=BF16)
    wfbd[0:40, 0] = Wf[:, 0].astype(BF16)
    wfbd[40, 0] = big
    wfbd[41, 0] = -big
    wfbd[64:104, 1] = Wf[:, 0].astype(BF16)
    wfbd[104, 1] = big
    wfbd[105, 1] = -big

    b2s = np.zeros((128, 1), dtype=f32)
    b2s[0:H2, 0] = b2
    b2s[64 : 64 + H2, 0] = b2

    return {
        "comb": comb,
        "v2d": v2d,
        "w2p": w2p,
        "wfbd": wfbd,
        "b2s": b2s,
        "ident": np.eye(128, dtype=BF16),
    }


def _postprocess(res_c):
    """outp [128, NG, 32]: cols 0:16 v-sums (q, jb), cols 16:32 row-0
    exp-sums ordered (jb, jj, i) -> [BC, D]."""
    op = np.asarray(res_c["outp"], dtype=np.float32)
    ov = op[:, :, 0:16].reshape(128, NG, 8, 2)
    osum = op[0, :, 16:32].reshape(NG, 2, 4, 2)  # (g, jb, jj, i)
    s = np.empty((NG, 16), dtype=np.float32)
    o = np.empty((NG, 16, D), dtype=np.float32)
    for jj in range(4):
        for i in range(2):
            for jb in range(2):
                bi = 4 * jj + 2 * i + jb
                s[:, bi] = osum[:, jb, jj, i]
                o[:, bi, :] = ov[jb * D : (jb + 1) * D, :, 2 * jj + i, jb].T
    s = np.where(s == 0.0, np.float32(1.0), s)
    return (o / s[:, :, None]).reshape(BC, D)


_LAST_RES = None  # stashed BassKernelResults (exec_time_ns etc.) for test harness


def kernel(q, k, v, mask, W1, b1, W2, b2, Wf, bf, **_):
    global _LAST_RES
    from concourse.bass_utils import run_bass_kernel_spmd

    q = np.asarray(q, dtype=np.float32)
    k = np.asarray(k, dtype=np.float32)
    v = np.asarray(v, dtype=np.float32)
    mask = np.asarray(mask)
    W1 = np.asarray(W1, dtype=np.float32)
    b1 = np.asarray(b1, dtype=np.float32)
    W2 = np.asarray(W2, dtype=np.float32)
    b2 = np.asarray(b2, dtype=np.float32)
    Wf = np.asarray(Wf, dtype=np.float32)

    nc = _get_nc()
    in_maps = []
    for c in range(NCORES):
        s = slice(c * BC, (c + 1) * BC)
        in_maps.append(_prep_core(q[s], k[s], v[s], mask[s], W1, b1, W2, b2, Wf))

    res = run_bass_kernel_spmd(nc, in_maps, list(range(NCORES)))
    _LAST_RES = res
    results = res.results

    out = np.empty((B, D), dtype=np.float32)
    for c in range(NCORES):
        out[c * BC : (c + 1) * BC] = _postprocess(results[c])
    return out


if __name__ == "__main__":
    rng = np.random.default_rng(0)
    inputs = {
        "q": rng.standard_normal((B, D), dtype=np.float32),
        "k": rng.standard_normal((B, T, D), dtype=np.float32),
        "v": rng.standard_normal((B, T, D), dtype=np.float32),
        "mask": rng.integers(0, 2, size=(B, T)).astype(np.int32),
        "W1": rng.standard_normal((4 * D, H1), dtype=np.float32) * 0.05,
        "b1": np.zeros(H1, np.float32),
        "W2": rng.standard_normal((H1, H2), dtype=np.float32) * 0.05,
        "b2": np.zeros(H2, np.float32),
        "Wf": rng.standard_normal((H2, 1), dtype=np.float32) * 0.05,
        "bf": np.zeros(1, np.float32),
    }
    out = kernel(**inputs)
    print(out.shape, out.dtype, np.abs(out).max())
